# revision 42
# baseline (speedup 1.0000x reference)
"""BlockGlobalAttentionProduct Trainium2 kernel.

Sharding: 24 (n,h) pairs across 8 cores, 3 per core. Each core, per (n,h):
  - dma_gather of interleaved [K|V] bf16 rows (256B) by local_idx / global_idx,
    split into 1024-index quarters (L/G interleaved) so compute starts after
    the first quarter; all indices preloaded in one tile at program start
  - PE transposes build K^T (d on partitions) for the score matmuls
  - scores^T computed per key tile (keys on partitions, queries on free dim);
    the local-window halo mask is folded into the matmul as 4 extra
    contraction rows (constant one-hot key patterns paired with -240 query
    rows), so exp() of masked slots is ~0 with no memsets
  - exp on ScalarE (scale=1/8 folded in) — ScalarE is the pacing engine
  - PV accumulated in ctx^T form (d+1 rows incl. sum-of-exp) in PSUM,
    interleaved with the score/exp packs per 512-query segment; emission is
    software-pipelined across (n,h) (next-unit transposes/packs fill the
    current unit's PV tail) so ScalarE and PE stay busy; a PE p-state warm-up
    burst precedes the first real matmuls
  - ctx^T stored bf16, DMA'd out in per-2-segment chunks; host does the final
    divide-by-denominator + transpose
"""

import sys

sys.path.insert(0, "/opt/trn_rl_repo")

import numpy as np
import ml_dtypes

import concourse.bacc as bacc
import concourse.mybir as mybir
from concourse import bass, tile, bass_utils, library_config

# problem shape (hardcoded per spec)
N, H, T, D = 2, 12, 4096, 64
NH = N * H            # 24
NCORES = 8
PER_CORE = NH // NCORES  # 3
NTILE = T // 128      # 32 key tiles per table
NSEG = 8              # query segments of 512
QW = 128 + T + 128    # qT halo width: cols [-128, 4224)

BF16 = mybir.dt.bfloat16
F32 = mybir.dt.float32
I16 = mybir.dt.int16

EXP = mybir.ActivationFunctionType.Exp


def _intervals(a0, width, s):
    """Pieces of window [a0, a0+width) mod T intersected with segment
    [512s, 512(s+1)). Yields (tile_col_offset, seg_col_offset, length)."""
    lo, hi = 512 * s, 512 * (s + 1)
    pieces = []
    a0 %= T
    if a0 + width <= T:
        pieces.append((a0, a0 + width, 0))
    else:
        pieces.append((a0, T, 0))
        pieces.append((0, (a0 + width) % T, T - a0))
    out = []
    for wa, wb, base in pieces:
        u, v = max(wa, lo), min(wb, hi)
        if u < v:
            out.append((base + (u - wa), u - lo, v - u))
    return out


def build_program():
    nc = bacc.Bacc("TRN2", target_bir_lowering=False, debug=False,
                   num_devices=NCORES)

    qTh = nc.dram_tensor("qTh", [PER_CORE, 68, QW], BF16, kind="ExternalInput")
    kvT = nc.dram_tensor("kv", [PER_CORE, T, 128], BF16, kind="ExternalInput")
    kmask_d = nc.dram_tensor("kmask", [4, T], BF16, kind="ExternalInput")
    gkT_d = nc.dram_tensor("gkT", [PER_CORE, 64, 64], BF16, kind="ExternalInput")
    # gv1[:, :, p, :]: [gv|1] rows zero-padded on the opposite 64-partition
    # half, so gtok PV can contract the full 128 partitions of the
    # column-paired expT layout (parity p selects which half is live).
    gv1_d = nc.dram_tensor("gv1", [PER_CORE, 128, 2, 65], BF16, kind="ExternalInput")
    idx_d = nc.dram_tensor("idx", [128, PER_CORE * 2 * 256], I16,
                           kind="ExternalInput")
    ident_d = nc.dram_tensor("ident", [128, 128], BF16, kind="ExternalInput")
    out_d = nc.dram_tensor("ctxT", [PER_CORE, 65, T], BF16, kind="ExternalOutput")

    with tile.TileContext(nc) as tc:
        with (
            tc.tile_pool(name="const", bufs=1) as constp,
            tc.tile_pool(name="land", bufs=2) as land,
            tc.tile_pool(name="kt", bufs=1) as ktp,
            tc.tile_pool(name="v1p", bufs=2) as v1p,
            tc.tile_pool(name="expp", bufs=2) as expp,
            tc.tile_pool(name="outp", bufs=1) as outp,
            tc.tile_pool(name="psL", bufs=1, space="PSUM") as psL,
            tc.tile_pool(name="psG", bufs=1, space="PSUM") as psG,
            tc.tile_pool(name="aux", bufs=2, space="PSUM") as auxp,
        ):
            ident = constp.tile([128, 128], BF16, tag="ident")
            idx_sb = constp.tile([128, PER_CORE * 2 * 256], I16, tag="idx")
            nc.sync.dma_start(idx_sb[:, 0:512], idx_d[:, 0:512])
            nc.sync.dma_start(idx_sb[:, 512:], idx_d[:, 512:])
            lib_i = nc.gpsimd.load_library(library_config.mlp)

            first_gather = [None]
            last_gather = [None]

            def emit_loads(i, q68=None):
                from concourse.tile_rust import add_dep_helper
                part = 0 if q68 is None else 1152
                if q68 is None:
                    q68 = land.tile([68, QW], BF16, tag="q")
                kvL = land.tile([128, NTILE, 128], BF16, tag="kvL")
                kvG = land.tile([128, NTILE, 128], BF16, tag="kvG")
                gkT = land.tile([64, 64], BF16, tag="gkT")
                gv1 = land.tile([128, 2, 65], BF16, tag="gv1")

                d1 = nc.sync.dma_start(q68[:, part:], qTh[i][:, part:])
                d2 = nc.sync.dma_start(gkT[:], gkT_d[i])
                d3 = nc.sync.dma_start(gv1[:], gv1_d[i])
                if last_gather[0] is not None:
                    for d in (d1, d2, d3):
                        add_dep_helper(d.ins, last_gather[0].ins,
                                       reason="prev gathers before next loads")
                gs = []
                for h in range(4):
                    for t, kv_sb in enumerate((kvL, kvG)):
                        base = (2 * i + t) * 256 + 64 * h
                        g = nc.gpsimd.dma_gather(
                            kv_sb[:, 8 * h:8 * h + 8, :], kvT[i],
                            idx_sb[:, base:base + 64], 1024, 1024, 128,
                            single_packet=False)
                        gs.append(g)
                if first_gather[0] is None:
                    add_dep_helper(lib_i.ins, gs[0].ins,
                                   reason="lib before gather")
                    first_gather[0] = gs[0]
                last_gather[0] = gs[-1]
                return dict(q68=q68, kvL=kvL, kvG=kvG, gkT=gkT, gv1=gv1)

            class NHState:
                """Per-(n,h) tiles + emission helpers."""

                def __init__(self, i, loads):
                    self.i = i
                    self.head_done = False
                    self.ktcopy = {}
                    self.q68 = loads["q68"]
                    self.kvL, self.kvG = loads["kvL"], loads["kvG"]
                    self.gkT, self.gv1 = loads["gkT"], loads["gv1"]
                    self.klT = ktp.tile([68, T], BF16, tag="klT")
                    self.kgT = ktp.tile([64, T], BF16, tag="kgT")
                    nc.sync.dma_start(self.klT[64:68, :], kmask_d[:])
                    self.v1L = v1p.tile([128, NTILE, 65], BF16, tag="v1L")
                    self.v1G = v1p.tile([128, NTILE, 65], BF16, tag="v1G")
                    self.expL = expp.tile([128, NTILE, 256], BF16, tag="expL")
                    self.expG = expp.tile([128, NTILE, 384], BF16, tag="expG")
                    self.expT = expp.tile([128, 4, 512], BF16, tag="expT")
                    self.ctx = outp.tile([65, T], BF16, tag="ctx")

                def v1copy(self, quarter):
                    # ordered after the same-quarter K^T copies so the
                    # scheduler's static DVE order can't hoist these (they
                    # wait on late gather quarters) ahead of the K^T copies
                    from concourse.tile_rust import add_dep_helper
                    sl = slice(8 * quarter, 8 * quarter + 8)
                    nc.gpsimd.memset(self.v1L[:, sl, 64:65], 1.0)
                    nc.gpsimd.memset(self.v1G[:, sl, 64:65], 1.0)
                    c1 = nc.vector.tensor_copy(self.v1L[:, sl, 0:64],
                                               self.kvL[:, sl, 64:128])
                    c2 = nc.vector.tensor_copy(self.v1G[:, sl, 0:64],
                                               self.kvG[:, sl, 64:128])
                    prev = self.ktcopy.get(("G", quarter))
                    if prev is not None:
                        add_dep_helper(c1.ins, prev.ins, reason="kt before v1")
                        add_dep_helper(c2.ins, prev.ins, reason="kt before v1")

                def trans(self, g, which, split=False):
                    # transpose key tiles [8g, 8g+8) into kT[0:64, 1024g:...];
                    # split=True copies per 4-tile half for lower latency
                    kv_sb, kT = ((self.kvL, self.klT) if which == "L"
                                 else (self.kvG, self.kgT))
                    tp = auxp.tile([64, 1024], BF16, tag="aux")
                    for half in range(2):
                        for pp in range(4 * half, 4 * half + 4):
                            c = 8 * g + pp
                            nc.tensor.transpose(
                                out=tp[:, pp * 128:(pp + 1) * 128],
                                in_=kv_sb[:, c, 0:64], identity=ident[:])
                        if split or half:
                            lo = 0 if not split else 512 * half
                            self.ktcopy[(which, g)] = nc.vector.tensor_copy(
                                kT[0:64, 1024 * g + lo:1024 * g + lo + (
                                    512 if split else 1024)],
                                tp[:, lo:lo + (512 if split else 1024)]
                                if split else tp[:])

                def Lp(self, p):
                    # local scores pack: key tiles 4p..4p+3, 256-query windows
                    st = psL.tile([128, 1024], F32, tag="pL")
                    for j in range(4):
                        c = 4 * p + j
                        rhs = self.q68[:, 64 + 128 * c:64 + 128 * c + 256]
                        nc.tensor.matmul(st[:, j * 256:(j + 1) * 256],
                                         self.klT[:, 128 * c:128 * c + 128],
                                         rhs, start=True, stop=True)
                    nc.scalar.activation(self.expL[:, 4 * p:4 * p + 4, :],
                                         st[:].rearrange("p (a b) -> p a b", b=256),
                                         EXP, scale=0.125)

                def Gp(self, p):
                    # global scores pack: key tiles 4p..4p+3, 384-query windows
                    st = psG.tile([128, 2048], F32, tag="pG")
                    for j in range(4):
                        t = 4 * p + j
                        rhs = self.q68[0:64, 128 * t:128 * t + 384]
                        nc.tensor.matmul(st[:, j * 512:j * 512 + 384],
                                         self.kgT[:, 128 * t:128 * t + 128],
                                         rhs, start=True, stop=True)
                    src = st[:].rearrange("p (a b) -> p a b", b=512)[:, :, 0:384]
                    nc.scalar.activation(self.expG[:, 4 * p:4 * p + 4, :], src,
                                         EXP, scale=0.125)

                def Tp(self, p):
                    # gtok scores: query segments 2p, 2p+1 at partition halves
                    st = auxp.tile([128, 512], F32, tag="aux")
                    for j in range(2):
                        g = 2 * p + j
                        nc.tensor.matmul(
                            st[j * 64:j * 64 + 64, 0:512], self.gkT[:],
                            self.q68[0:64, 128 + 512 * g:128 + 512 * g + 512],
                            start=True, stop=True, tile_position=(0, j * 64))
                    nc.scalar.activation(self.expT[:, p, :], st[:],
                                         EXP, scale=0.125)

                def PV(self, s):
                    acc = auxp.tile([65, 512], F32, tag="aux")
                    mms = []
                    late = []
                    # gtok initializes the whole segment (full-128 contraction;
                    # the inactive parity half of gv1 is zero)
                    mms.append((self.gv1[:, s % 2, :],
                                self.expT[:, s // 2, 0:512], 0, 512))
                    # tiles in pack s+1 (exp lands last) go at the end of the
                    # accumulation group so PE can start the rest sooner
                    newest = range(4 * (s + 1), 4 * (s + 2))
                    for c in range(NTILE):
                        for (tc_, sc, ln) in _intervals((2 * c - 1) * 64, 256, s):
                            (late if c in newest else mms).append(
                                (self.v1L[:, c, :],
                                 self.expL[:, c, tc_:tc_ + ln], sc, ln))
                    for t in range(NTILE):
                        for (tc_, sc, ln) in _intervals((t - 1) * 128, 384, s):
                            (late if t in newest else mms).append(
                                (self.v1G[:, t, :],
                                 self.expG[:, t, tc_:tc_ + ln], sc, ln))
                    mms += late
                    for mi, (lhsT, rhs, sc, ln) in enumerate(mms):
                        nc.tensor.matmul(acc[:, sc:sc + ln], lhsT, rhs,
                                         start=(mi == 0),
                                         stop=(mi == len(mms) - 1),
                                         skip_group_check=True)
                    nc.vector.tensor_copy(self.ctx[:, 512 * s:512 * (s + 1)],
                                          acc[:])

                def out_chunk(self, c):
                    # chunk 0: seg 0; 1: segs 1-2; 2: 3-4; 3: 5-6; 4: seg 7
                    lo, hi = [(0, 512), (512, 1536), (1536, 2560),
                              (2560, 3584), (3584, 4096)][c]
                    nc.sync.dma_start(out_d[self.i][:, lo:hi],
                                      self.ctx[:, lo:hi])

            # ---- software-pipelined emission across the 3 (n,h) units ----
            # PE p-state warm-up: ~3us of throwaway matmuls on the idx tile
            # (bitcast to bf16 = tiny denormals) so the first real matmuls
            # run at full clock instead of the cold 0.65GHz p-state
            warm = auxp.tile([128, 512], F32, tag="aux")
            idxbf = idx_sb[:].bitcast(BF16)
            for w in range(7):
                nc.tensor.matmul(warm[:], idxbf[:, 0:128], idxbf[:, 0:512],
                                 start=True, stop=True)

            # nh0 head emitted in gather-arrival order: q68-only gtok packs
            # first, then each quarter's L work, then its G work
            cur = NHState(0, emit_loads(0))
            nc.sync.dma_start(ident[:], ident_d[:])
            cur.Tp(0); cur.Tp(1); cur.Tp(2); cur.Tp(3)
            cur.trans(0, "L", split=True)
            cur.Lp(0); cur.Lp(1)
            cur.trans(0, "G", split=True); cur.v1copy(0)
            cur.Gp(0); cur.Gp(1)
            cur.trans(1, "L")
            cur.Lp(2)
            cur.trans(1, "G"); cur.v1copy(1)
            cur.Gp(2)
            cur.trans(2, "L")
            cur.trans(2, "G"); cur.v1copy(2)
            cur.head_done = True

            for i in range(PER_CORE):
                if cur.head_done is False:
                    cur.trans(2, "L"); cur.trans(2, "G"); cur.v1copy(2)
                    cur.Lp(3); cur.Gp(3)
                cur.PV(1)
                cur.PV(2)
                # logical-time floor keeps these DMAs from stealing the DMA
                # engines ahead of this nh's gather stream
                if i + 1 < PER_CORE:
                    with tc.tile_wait_until(0.010 + 0.0265 * i):
                        loads_next = emit_loads(i + 1)
                else:
                    loads_next = None
                cur.out_chunk(1)
                cur.Lp(4); cur.Gp(4)
                if not cur.head_done:
                    cur.Tp(2)
                cur.PV(3)
                cur.trans(3, "L"); cur.trans(3, "G"); cur.v1copy(3)
                cur.Lp(5); cur.Gp(5)
                cur.PV(4)
                cur.out_chunk(2)
                cur.Lp(6); cur.Gp(6)
                if not cur.head_done:
                    cur.Tp(3)
                cur.PV(5)
                cur.Lp(7); cur.Gp(7)
                nxt = NHState(i + 1, loads_next) if loads_next else None
                if nxt is not None:
                    nxt.trans(0, "L"); nxt.trans(0, "G"); nxt.v1copy(0)
                cur.PV(6)
                cur.out_chunk(3)
                if nxt is not None:
                    nxt.Lp(0); nxt.Gp(0); nxt.Tp(0)
                cur.PV(7)
                if nxt is not None:
                    nxt.Lp(1); nxt.Gp(1)
                    nxt.trans(1, "L"); nxt.trans(1, "G"); nxt.v1copy(1)
                cur.PV(0)
                cur.out_chunk(4)
                cur.out_chunk(0)
                if nxt is not None:
                    nxt.Lp(2); nxt.Gp(2); nxt.Tp(1)
                    nxt.trans(2, "L"); nxt.trans(2, "G"); nxt.v1copy(2)
                    nxt.Lp(3); nxt.Gp(3)
                cur = nxt

    nc.compile()
    return nc


_CACHED = None


def _get_program():
    global _CACHED
    if _CACHED is None:
        _CACHED = build_program()
    return _CACHED


def _prep_core_inputs(q, k, v, gk, gv, lidx, gidx, pairs):
    """Build one core's input dict for its list of (n,h) pairs."""
    bf = ml_dtypes.bfloat16
    qTh = np.empty((PER_CORE, 68, QW), dtype=bf)
    kv = np.empty((PER_CORE, T, 128), dtype=bf)
    gkT = np.empty((PER_CORE, 64, 64), dtype=bf)
    gv1 = np.zeros((PER_CORE, 128, 2, 65), dtype=bf)
    idx = np.empty((128, PER_CORE * 2 * 256), dtype=np.int16)
    # query-side mask rows: -240 on the quadrant of (col-64)//64 mod 4 that
    # pairs with each kmask row (so exp(0.125*(s-240)) ~ 0 on masked slots)
    j = np.arange(QW)
    quad = ((j - 64) // 64) % 4
    qmask = np.zeros((4, QW), np.float32)
    for r, qd in enumerate((0, 3, 2, 1)):
        qmask[r, quad == qd] = -240.0
    for s, (n, h) in enumerate(pairs):
        qt = np.ascontiguousarray(q[n, h].T)            # (64, T) f32
        qth = np.concatenate([qt[:, T - 128:], qt, qt[:, :128]], axis=1)
        qTh[s, 0:64] = qth.astype(bf)
        qTh[s, 64:68] = qmask.astype(bf)
        kv[s, :, 0:64] = k[n, h].astype(bf)
        kv[s, :, 64:128] = v[n, h].astype(bf)
        gkT[s] = np.ascontiguousarray(gk[n, h].T).astype(bf)
        g1 = np.concatenate([gv[n, h], np.ones((64, 1), np.float32)],
                            axis=1).astype(bf)
        gv1[s, 0:64, 0] = g1      # parity 0: top half live
        gv1[s, 64:128, 1] = g1    # parity 1: bottom half live
        for t, src in ((0, lidx), (1, gidx)):
            ix = src[n, h, :, 0].astype(np.int16)       # (T,)
            base = (2 * s + t) * 256
            idx[:, base:base + 256] = np.tile(
                ix.reshape(T // 16, 16).T, (8, 1))
    ident = np.eye(128, dtype=bf)
    # key-side mask rows: one-hot (period 256) selecting (key-half, c-parity)
    m = np.arange(T) % 256
    kmask = np.stack([(m >= 64) & (m < 128), m < 64,
                      m >= 192, (m >= 128) & (m < 192)]).astype(np.float32)
    return {"qTh": qTh, "kv": kv, "gkT": gkT, "gv1": gv1, "kmask":
            kmask.astype(bf), "idx": idx, "ident": ident}


def kernel(query_layer, key_layer, value_layer, attention_mask, local_idx,
           global_idx, global_key, global_value, global_mask):
    # attention_mask / global_mask are all-zero in this problem's input spec;
    # they contribute nothing to the scores and are not shipped to the device.
    q = np.asarray(query_layer, np.float32)
    k = np.asarray(key_layer, np.float32)
    v = np.asarray(value_layer, np.float32)
    gk = np.asarray(global_key, np.float32)
    gv = np.asarray(global_value, np.float32)
    li = np.asarray(local_idx)
    gi = np.asarray(global_idx)

    nc = _get_program()
    in_maps = []
    for m in range(NCORES):
        pairs = [((3 * m + s) // H, (3 * m + s) % H) for s in range(PER_CORE)]
        in_maps.append(_prep_core_inputs(q, k, v, gk, gv, li, gi, pairs))
    res = bass_utils.run_bass_kernel_spmd(nc, in_maps, core_ids=list(range(NCORES)))

    out = np.empty((N, H, T, D), np.float32)
    for m in range(NCORES):
        ctxT = np.asarray(res.results[m]["ctxT"], dtype=np.float32)  # (3, 65, T)
        for s in range(PER_CORE):
            n, h = (3 * m + s) // H, (3 * m + s) % H
            out[n, h] = (ctxT[s, :64] / ctxT[s, 64:65]).T
    return out


# revision 43
# speedup vs baseline: 1.0593x; 1.0593x over previous
"""BlockGlobalAttentionProduct Trainium2 kernel.

Sharding: 24 (n,h) pairs across 8 cores, 3 per core. Each core, per (n,h):
  - dma_gather of interleaved [K|V] bf16 rows (256B) by local_idx / global_idx,
    split into 1024-index quarters (L/G interleaved) so compute starts after
    the first quarter; all indices preloaded in one tile at program start
  - PE transposes build K^T (d on partitions) for the score matmuls
  - scores^T computed per key tile (keys on partitions, queries on free dim);
    the local-window halo mask is folded into the matmul as 4 extra
    contraction rows (constant one-hot key patterns paired with -240 query
    rows), so exp() of masked slots is ~0 with no memsets
  - exp on ScalarE (scale=1/8 folded in) — ScalarE is the pacing engine
  - PV accumulated in ctx^T form (d+1 rows incl. sum-of-exp) in PSUM,
    interleaved with the score/exp packs per 512-query segment; emission is
    software-pipelined across (n,h) (next-unit transposes/packs fill the
    current unit's PV tail) so ScalarE and PE stay busy; a PE p-state warm-up
    burst precedes the first real matmuls
  - ctx^T stored bf16, DMA'd out in per-2-segment chunks; host does the final
    divide-by-denominator + transpose
"""

import sys

sys.path.insert(0, "/opt/trn_rl_repo")

import numpy as np
import ml_dtypes

import concourse.bacc as bacc
import concourse.mybir as mybir
from concourse import bass, tile, bass_utils, library_config

# problem shape (hardcoded per spec)
N, H, T, D = 2, 12, 4096, 64
NH = N * H            # 24
NCORES = 8
PER_CORE = NH // NCORES  # 3
NTILE = T // 128      # 32 key tiles per table
NSEG = 8              # query segments of 512
QW = 128 + T + 128    # qT halo width: cols [-128, 4224)

BF16 = mybir.dt.bfloat16
F32 = mybir.dt.float32
I16 = mybir.dt.int16

EXP = mybir.ActivationFunctionType.Exp


def _intervals(a0, width, s):
    """Pieces of window [a0, a0+width) mod T intersected with segment
    [512s, 512(s+1)). Yields (tile_col_offset, seg_col_offset, length)."""
    lo, hi = 512 * s, 512 * (s + 1)
    pieces = []
    a0 %= T
    if a0 + width <= T:
        pieces.append((a0, a0 + width, 0))
    else:
        pieces.append((a0, T, 0))
        pieces.append((0, (a0 + width) % T, T - a0))
    out = []
    for wa, wb, base in pieces:
        u, v = max(wa, lo), min(wb, hi)
        if u < v:
            out.append((base + (u - wa), u - lo, v - u))
    return out


def build_program():
    nc = bacc.Bacc("TRN2", target_bir_lowering=False, debug=False,
                   num_devices=NCORES)

    qTh = nc.dram_tensor("qTh", [PER_CORE, 68, QW], BF16, kind="ExternalInput")
    kvT = nc.dram_tensor("kv", [PER_CORE, T, 128], BF16, kind="ExternalInput")
    kmask_d = nc.dram_tensor("kmask", [4, T], BF16, kind="ExternalInput")
    gkT_d = nc.dram_tensor("gkT", [PER_CORE, 64, 64], BF16, kind="ExternalInput")
    # gv1[:, :, p, :]: [gv|1] rows zero-padded on the opposite 64-partition
    # half, so gtok PV can contract the full 128 partitions of the
    # column-paired expT layout (parity p selects which half is live).
    gv1_d = nc.dram_tensor("gv1", [PER_CORE, 128, 2, 65], BF16, kind="ExternalInput")
    idx_d = nc.dram_tensor("idx", [128, PER_CORE * 2 * 256], I16,
                           kind="ExternalInput")
    ident_d = nc.dram_tensor("ident", [128, 128], BF16, kind="ExternalInput")
    out_d = nc.dram_tensor("ctxT", [PER_CORE, 65, T], BF16, kind="ExternalOutput")

    with tile.TileContext(nc) as tc:
        with (
            tc.tile_pool(name="const", bufs=1) as constp,
            tc.tile_pool(name="land", bufs=2) as land,
            tc.tile_pool(name="kt", bufs=1) as ktp,
            tc.tile_pool(name="v1p", bufs=2) as v1p,
            tc.tile_pool(name="expp", bufs=2) as expp,
            tc.tile_pool(name="outp", bufs=1) as outp,
            tc.tile_pool(name="psL", bufs=1, space="PSUM") as psL,
            tc.tile_pool(name="psG", bufs=1, space="PSUM") as psG,
            tc.tile_pool(name="aux", bufs=2, space="PSUM") as auxp,
        ):
            ident = constp.tile([128, 128], BF16, tag="ident")
            idx_sb = constp.tile([128, PER_CORE * 2 * 256], I16, tag="idx")
            nc.sync.dma_start(idx_sb[:, 0:512], idx_d[:, 0:512])
            nc.sync.dma_start(idx_sb[:, 512:], idx_d[:, 512:])
            lib_i = nc.gpsimd.load_library(library_config.mlp)

            first_gather = [None]
            last_gather = [None]

            def emit_loads(i, q68=None):
                from concourse.tile_rust import add_dep_helper
                part = 0 if q68 is None else 1152
                if q68 is None:
                    q68 = land.tile([68, QW], BF16, tag="q")
                kvL = land.tile([128, NTILE, 128], BF16, tag="kvL")
                kvG = land.tile([128, NTILE, 128], BF16, tag="kvG")
                gkT = land.tile([64, 64], BF16, tag="gkT")
                gv1 = land.tile([128, 2, 65], BF16, tag="gv1")

                d1 = nc.sync.dma_start(q68[:, part:], qTh[i][:, part:])
                d2 = nc.sync.dma_start(gkT[:], gkT_d[i])
                d3 = nc.sync.dma_start(gv1[:], gv1_d[i])
                if last_gather[0] is not None:
                    for d in (d1, d2, d3):
                        add_dep_helper(d.ins, last_gather[0].ins,
                                       reason="prev gathers before next loads")
                gs = []
                for h in range(4):
                    for t, kv_sb in enumerate((kvL, kvG)):
                        base = (2 * i + t) * 256 + 64 * h
                        g = nc.gpsimd.dma_gather(
                            kv_sb[:, 8 * h:8 * h + 8, :], kvT[i],
                            idx_sb[:, base:base + 64], 1024, 1024, 128,
                            single_packet=False)
                        gs.append(g)
                if first_gather[0] is None:
                    add_dep_helper(lib_i.ins, gs[0].ins,
                                   reason="lib before gather")
                    first_gather[0] = gs[0]
                last_gather[0] = gs[-1]
                return dict(q68=q68, kvL=kvL, kvG=kvG, gkT=gkT, gv1=gv1)

            class NHState:
                """Per-(n,h) tiles + emission helpers."""

                def __init__(self, i, loads):
                    self.i = i
                    self.head_done = False
                    self.ktcopy = {}
                    self.q68 = loads["q68"]
                    self.kvL, self.kvG = loads["kvL"], loads["kvG"]
                    self.gkT, self.gv1 = loads["gkT"], loads["gv1"]
                    self.klT = ktp.tile([68, T], BF16, tag="klT")
                    self.kgT = ktp.tile([64, T], BF16, tag="kgT")
                    nc.sync.dma_start(self.klT[64:68, :], kmask_d[:])
                    self.v1L = v1p.tile([128, NTILE, 65], BF16, tag="v1L")
                    self.v1G = v1p.tile([128, NTILE, 65], BF16, tag="v1G")
                    self.expL = expp.tile([128, NTILE, 256], BF16, tag="expL")
                    self.expG = expp.tile([128, NTILE, 384], BF16, tag="expG")
                    self.expT = expp.tile([128, 4, 512], BF16, tag="expT")
                    self.ctx = outp.tile([65, T], BF16, tag="ctx")

                def v1copy(self, quarter):
                    # ordered after the same-quarter K^T copies so the
                    # scheduler's static DVE order can't hoist these (they
                    # wait on late gather quarters) ahead of the K^T copies
                    from concourse.tile_rust import add_dep_helper
                    sl = slice(8 * quarter, 8 * quarter + 8)
                    nc.gpsimd.memset(self.v1L[:, sl, 64:65], 1.0)
                    nc.gpsimd.memset(self.v1G[:, sl, 64:65], 1.0)
                    c1 = nc.vector.tensor_copy(self.v1L[:, sl, 0:64],
                                               self.kvL[:, sl, 64:128])
                    c2 = nc.vector.tensor_copy(self.v1G[:, sl, 0:64],
                                               self.kvG[:, sl, 64:128])
                    prev = self.ktcopy.get(("G", quarter))
                    if prev is not None:
                        add_dep_helper(c1.ins, prev.ins, reason="kt before v1")
                        add_dep_helper(c2.ins, prev.ins, reason="kt before v1")

                def trans(self, g, which, split=False):
                    # transpose key tiles [8g, 8g+8) into kT[0:64, 1024g:...];
                    # split=True copies per 4-tile half for lower latency
                    kv_sb, kT = ((self.kvL, self.klT) if which == "L"
                                 else (self.kvG, self.kgT))
                    tp = auxp.tile([64, 1024], BF16, tag="aux")
                    for half in range(2):
                        for pp in range(4 * half, 4 * half + 4):
                            c = 8 * g + pp
                            nc.tensor.transpose(
                                out=tp[:, pp * 128:(pp + 1) * 128],
                                in_=kv_sb[:, c, 0:64], identity=ident[:])
                        if split or half:
                            lo = 0 if not split else 512 * half
                            self.ktcopy[(which, g)] = nc.vector.tensor_copy(
                                kT[0:64, 1024 * g + lo:1024 * g + lo + (
                                    512 if split else 1024)],
                                tp[:, lo:lo + (512 if split else 1024)]
                                if split else tp[:])

                def Lp(self, p):
                    # local scores pack: key tiles 4p..4p+3, 256-query windows
                    st = psL.tile([128, 1024], F32, tag="pL")
                    for j in range(4):
                        c = 4 * p + j
                        rhs = self.q68[:, 64 + 128 * c:64 + 128 * c + 256]
                        nc.tensor.matmul(st[:, j * 256:(j + 1) * 256],
                                         self.klT[:, 128 * c:128 * c + 128],
                                         rhs, start=True, stop=True)
                    nc.scalar.activation(self.expL[:, 4 * p:4 * p + 4, :],
                                         st[:].rearrange("p (a b) -> p a b", b=256),
                                         EXP, scale=0.125)

                def Gp(self, p):
                    # global scores pack: key tiles 4p..4p+3, 384-query windows
                    st = psG.tile([128, 2048], F32, tag="pG")
                    for j in range(4):
                        t = 4 * p + j
                        rhs = self.q68[0:64, 128 * t:128 * t + 384]
                        nc.tensor.matmul(st[:, j * 512:j * 512 + 384],
                                         self.kgT[:, 128 * t:128 * t + 128],
                                         rhs, start=True, stop=True)
                    src = st[:].rearrange("p (a b) -> p a b", b=512)[:, :, 0:384]
                    nc.scalar.activation(self.expG[:, 4 * p:4 * p + 4, :], src,
                                         EXP, scale=0.125)

                def Tp(self, p):
                    # gtok scores: query segments 2p, 2p+1 at partition halves
                    st = auxp.tile([128, 512], F32, tag="aux")
                    for j in range(2):
                        g = 2 * p + j
                        nc.tensor.matmul(
                            st[j * 64:j * 64 + 64, 0:512], self.gkT[:],
                            self.q68[0:64, 128 + 512 * g:128 + 512 * g + 512],
                            start=True, stop=True, tile_position=(0, j * 64))
                    nc.scalar.activation(self.expT[:, p, :], st[:],
                                         EXP, scale=0.125)

                def PV(self, s):
                    acc = auxp.tile([65, 512], F32, tag="aux")
                    mms = []
                    late = []
                    # gtok initializes the whole segment (full-128 contraction;
                    # the inactive parity half of gv1 is zero)
                    mms.append((self.gv1[:, s % 2, :],
                                self.expT[:, s // 2, 0:512], 0, 512))
                    # tiles in pack s+1 (exp lands last) go at the end of the
                    # accumulation group so PE can start the rest sooner
                    newest = range(4 * (s + 1), 4 * (s + 2))
                    for c in range(NTILE):
                        for (tc_, sc, ln) in _intervals((2 * c - 1) * 64, 256, s):
                            (late if c in newest else mms).append(
                                (self.v1L[:, c, :],
                                 self.expL[:, c, tc_:tc_ + ln], sc, ln))
                    for t in range(NTILE):
                        for (tc_, sc, ln) in _intervals((t - 1) * 128, 384, s):
                            (late if t in newest else mms).append(
                                (self.v1G[:, t, :],
                                 self.expG[:, t, tc_:tc_ + ln], sc, ln))
                    mms += late
                    for mi, (lhsT, rhs, sc, ln) in enumerate(mms):
                        nc.tensor.matmul(acc[:, sc:sc + ln], lhsT, rhs,
                                         start=(mi == 0),
                                         stop=(mi == len(mms) - 1),
                                         skip_group_check=True)
                    nc.vector.tensor_copy(self.ctx[:, 512 * s:512 * (s + 1)],
                                          acc[:])

                def out_chunk(self, c):
                    # chunk 0: seg 0; 1: segs 1-2; 2: 3-4; 3: 5-6; 4: seg 7
                    lo, hi = [(0, 512), (512, 1536), (1536, 2560),
                              (2560, 3584), (3584, 4096)][c]
                    nc.sync.dma_start(out_d[self.i][:, lo:hi],
                                      self.ctx[:, lo:hi])

            # ---- software-pipelined emission across the 3 (n,h) units ----
            # PE p-state warm-up: ~3us of throwaway matmuls on the idx tile
            # (bitcast to bf16 = tiny denormals) so the first real matmuls
            # run at full clock instead of the cold 0.65GHz p-state
            warm = auxp.tile([128, 512], F32, tag="aux")
            idxbf = idx_sb[:].bitcast(BF16)
            for w in range(7):
                nc.tensor.matmul(warm[:], idxbf[:, 0:128], idxbf[:, 0:512],
                                 start=True, stop=True)

            # nh0 head emitted in gather-arrival order: q68-only gtok packs
            # first, then each quarter's L work, then its G work
            cur = NHState(0, emit_loads(0))
            nc.sync.dma_start(ident[:], ident_d[:])
            cur.Tp(0); cur.Tp(1); cur.Tp(2); cur.Tp(3)
            cur.trans(0, "L", split=True)
            cur.Lp(0); cur.Lp(1)
            cur.trans(0, "G", split=True); cur.v1copy(0)
            cur.Gp(0); cur.Gp(1)
            cur.trans(1, "L")
            cur.Lp(2)
            cur.trans(1, "G"); cur.v1copy(1)
            cur.Gp(2)
            cur.trans(2, "L")
            cur.trans(2, "G"); cur.v1copy(2)
            cur.head_done = True

            for i in range(PER_CORE):
                if cur.head_done is False:
                    cur.trans(2, "L"); cur.trans(2, "G"); cur.v1copy(2)
                cur.PV(1)
                cur.Lp(3); cur.Gp(3)
                cur.PV(2)
                # logical-time floor keeps these DMAs from stealing the DMA
                # engines ahead of this nh's gather stream
                if i + 1 < PER_CORE:
                    with tc.tile_wait_until(0.010 + 0.0265 * i):
                        loads_next = emit_loads(i + 1)
                else:
                    loads_next = None
                cur.out_chunk(1)
                cur.Lp(4); cur.Gp(4)
                if not cur.head_done:
                    cur.Tp(2)
                cur.PV(3)
                cur.trans(3, "L"); cur.trans(3, "G"); cur.v1copy(3)
                cur.Lp(5); cur.Gp(5)
                cur.PV(4)
                cur.out_chunk(2)
                cur.Lp(6); cur.Gp(6)
                if not cur.head_done:
                    cur.Tp(3)
                cur.PV(5)
                cur.Lp(7); cur.Gp(7)
                nxt = NHState(i + 1, loads_next) if loads_next else None
                if nxt is not None:
                    nxt.trans(0, "L"); nxt.trans(0, "G"); nxt.v1copy(0)
                cur.PV(6)
                cur.out_chunk(3)
                if nxt is not None:
                    nxt.Lp(0); nxt.Gp(0); nxt.Tp(0)
                cur.PV(7)
                if nxt is not None:
                    nxt.Lp(1); nxt.Gp(1)
                    nxt.trans(1, "L"); nxt.trans(1, "G"); nxt.v1copy(1)
                cur.PV(0)
                cur.out_chunk(4)
                cur.out_chunk(0)
                if nxt is not None:
                    nxt.Lp(2); nxt.Gp(2); nxt.Tp(1)
                cur = nxt

    nc.compile()
    return nc


_CACHED = None


def _get_program():
    global _CACHED
    if _CACHED is None:
        _CACHED = build_program()
    return _CACHED


def _prep_core_inputs(q, k, v, gk, gv, lidx, gidx, pairs):
    """Build one core's input dict for its list of (n,h) pairs."""
    bf = ml_dtypes.bfloat16
    qTh = np.empty((PER_CORE, 68, QW), dtype=bf)
    kv = np.empty((PER_CORE, T, 128), dtype=bf)
    gkT = np.empty((PER_CORE, 64, 64), dtype=bf)
    gv1 = np.zeros((PER_CORE, 128, 2, 65), dtype=bf)
    idx = np.empty((128, PER_CORE * 2 * 256), dtype=np.int16)
    # query-side mask rows: -240 on the quadrant of (col-64)//64 mod 4 that
    # pairs with each kmask row (so exp(0.125*(s-240)) ~ 0 on masked slots)
    j = np.arange(QW)
    quad = ((j - 64) // 64) % 4
    qmask = np.zeros((4, QW), np.float32)
    for r, qd in enumerate((0, 3, 2, 1)):
        qmask[r, quad == qd] = -240.0
    for s, (n, h) in enumerate(pairs):
        qt = np.ascontiguousarray(q[n, h].T)            # (64, T) f32
        qth = np.concatenate([qt[:, T - 128:], qt, qt[:, :128]], axis=1)
        qTh[s, 0:64] = qth.astype(bf)
        qTh[s, 64:68] = qmask.astype(bf)
        kv[s, :, 0:64] = k[n, h].astype(bf)
        kv[s, :, 64:128] = v[n, h].astype(bf)
        gkT[s] = np.ascontiguousarray(gk[n, h].T).astype(bf)
        g1 = np.concatenate([gv[n, h], np.ones((64, 1), np.float32)],
                            axis=1).astype(bf)
        gv1[s, 0:64, 0] = g1      # parity 0: top half live
        gv1[s, 64:128, 1] = g1    # parity 1: bottom half live
        for t, src in ((0, lidx), (1, gidx)):
            ix = src[n, h, :, 0].astype(np.int16)       # (T,)
            base = (2 * s + t) * 256
            idx[:, base:base + 256] = np.tile(
                ix.reshape(T // 16, 16).T, (8, 1))
    ident = np.eye(128, dtype=bf)
    # key-side mask rows: one-hot (period 256) selecting (key-half, c-parity)
    m = np.arange(T) % 256
    kmask = np.stack([(m >= 64) & (m < 128), m < 64,
                      m >= 192, (m >= 128) & (m < 192)]).astype(np.float32)
    return {"qTh": qTh, "kv": kv, "gkT": gkT, "gv1": gv1, "kmask":
            kmask.astype(bf), "idx": idx, "ident": ident}


def kernel(query_layer, key_layer, value_layer, attention_mask, local_idx,
           global_idx, global_key, global_value, global_mask):
    # attention_mask / global_mask are all-zero in this problem's input spec;
    # they contribute nothing to the scores and are not shipped to the device.
    q = np.asarray(query_layer, np.float32)
    k = np.asarray(key_layer, np.float32)
    v = np.asarray(value_layer, np.float32)
    gk = np.asarray(global_key, np.float32)
    gv = np.asarray(global_value, np.float32)
    li = np.asarray(local_idx)
    gi = np.asarray(global_idx)

    nc = _get_program()
    in_maps = []
    for m in range(NCORES):
        pairs = [((3 * m + s) // H, (3 * m + s) % H) for s in range(PER_CORE)]
        in_maps.append(_prep_core_inputs(q, k, v, gk, gv, li, gi, pairs))
    res = bass_utils.run_bass_kernel_spmd(nc, in_maps, core_ids=list(range(NCORES)))

    out = np.empty((N, H, T, D), np.float32)
    for m in range(NCORES):
        ctxT = np.asarray(res.results[m]["ctxT"], dtype=np.float32)  # (3, 65, T)
        for s in range(PER_CORE):
            n, h = (3 * m + s) // H, (3 * m + s) % H
            out[n, h] = (ctxT[s, :64] / ctxT[s, 64:65]).T
    return out
